# revision 3
# baseline (speedup 1.0000x reference)
"""KANConv2D Trainium2 kernel (8 NeuronCores, data-parallel over batch).

Math: out = conv(x, kernel) + exp(-gamma * d) + bias, where
  d[n,f]  = pn[n] + cn[f] - 2*pc[n,f]
  pc      = conv(x, control_points)      (patches @ control_points)
  pn[n]   = sum of x^2 over the 3x3xC patch
  gamma   = 1 / (2 * mean(d))            (global mean -> AllReduce)

Device strategy per core (4 images):
  - host pre-transposes/pads the shard to [C=64, img, 66, 66] and stacks x^2
    on SBUF partitions 64..127, so q := pc - pn/2 falls out of ONE 9-tap
    accumulated matmul group per 512-pixel block:
      lhsT_tap = [[cp_tap (64x128)], [-1/2 (64x128)]]  (K=128)
  - conv runs as its own 9-tap group (K=64) in phase B.
  - matmuls use float32r (1-pass FP22 multiply, fp32 PSUM accumulate).
  - Sum(q) per core -> AllReduce -> gamma -> exp epilogue on ACT engine.
"""

import os
import sys

import numpy as np

for _p in ("/opt/trn_rl_repo", "/root/.axon_site/_ro/trn_rl_repo"):
    if os.path.isdir(_p) and _p not in sys.path:
        sys.path.insert(0, _p)

import concourse.bacc as bacc
import concourse.tile as tile
from concourse import mybir
from concourse.bass_utils import run_bass_kernel_spmd


def _ensure_ntff_hook():
    """bass_utils imports antenv.axon_hooks when tracing under axon; this
    image's antenv lacks that module. Provide it and install the ctypes
    NTFF hook so BASS_TRACE=1 yields exec_time_ns."""
    import types
    try:
        from antenv.axon_hooks import get_axon_ntff_profile_hook  # noqa: F401
        return
    except ImportError:
        pass
    try:
        import antenv
        mod = types.ModuleType("antenv.axon_hooks")
        _state = {"hook": None}
        mod.set_axon_ntff_profile_hook = lambda h: _state.__setitem__("hook", h)
        mod.get_axon_ntff_profile_hook = lambda: _state["hook"]
        sys.modules["antenv.axon_hooks"] = mod
        antenv.axon_hooks = mod
        try:
            from trn_agent_boot.trn_boot import _ntff_profile_via_ctypes
            so = "/opt/axon/libaxon_pjrt.so"
            if os.path.exists(so):
                mod.set_axon_ntff_profile_hook(_ntff_profile_via_ctypes(so))
        except Exception:
            pass
    except Exception:
        pass


_ensure_ntff_hook()

B, H, W, C, F = 32, 64, 64, 64, 128
KH = KW = 3
N_CORES = 8
IMGS = B // N_CORES          # 4 images per core
HP, WP = H + 2, W + 2        # 66 padded
ROWS_PER_BLK = 8
BLK = ROWS_PER_BLK * W       # 512 pixels per block
NBLK = IMGS * (H // ROWS_PER_BLK)   # 32 blocks per core
PIX = IMGS * H * W           # 16384 pixels per core
NTOT = B * H * W             # 131072 pixels total

F32 = mybir.dt.float32
F32R = mybir.dt.float32r

TAPS = [(kh, kw) for kh in range(KH) for kw in range(KW)]

LAST_EXEC_TIME_NS = None


def _r(ap):
    return ap.bitcast(F32R)


def _build(offset_const: float, scale_const: float):
    """offset_const = 2*sum(cn)/F ; scale_const = -4/(NTOT*F).
    gamma = 1 / (offset_const + scale_const * sum_q_total)."""
    nc = bacc.Bacc("TRN2", target_bir_lowering=False, debug=False,
                   num_devices=N_CORES)
    xx = nc.dram_tensor("xx", [128, IMGS, HP, WP], F32R, kind="ExternalInput")
    convw = nc.dram_tensor("convw", [64, 9 * F], F32R, kind="ExternalInput")
    qw = nc.dram_tensor("qw", [128, 9 * F], F32R, kind="ExternalInput")
    cnneg = nc.dram_tensor("cnneg", [128, 1], F32, kind="ExternalInput")
    biasf = nc.dram_tensor("biasf", [128, 1], F32, kind="ExternalInput")
    out = nc.dram_tensor("out", [128, PIX], F32, kind="ExternalOutput")

    with tile.TileContext(nc) as tc:
        with (
            tc.tile_pool(name="xp", bufs=1) as xp,
            tc.tile_pool(name="wp", bufs=1) as wp,
            tc.tile_pool(name="qs", bufs=1) as qs,
            tc.tile_pool(name="st", bufs=3) as st,
            tc.tile_pool(name="psq", bufs=2, space="PSUM") as psq,
            tc.tile_pool(name="psc", bufs=4, space="PSUM") as psc,
            tc.tile_pool(name="pss", bufs=1, space="PSUM") as pss,
            tc.tile_pool(name="dr", bufs=1, space="DRAM") as dr,
        ):
            # ---- loads ----
            x_t = []
            for i in range(IMGS):
                t = xp.tile([128, HP, WP], F32R, tag=f"x{i}")
                nc.sync.dma_start(out=t, in_=xx[:, i])
                x_t.append(t)
            cw = wp.tile([64, 9 * F], F32R, tag="cw")
            nc.sync.dma_start(out=cw, in_=convw[:])
            qwt = wp.tile([128, 9 * F], F32R, tag="qw")
            nc.sync.dma_start(out=qwt, in_=qw[:])
            cnn = wp.tile([128, 1], F32, tag="cnn")
            nc.sync.dma_start(out=cnn, in_=cnneg[:])
            bft = wp.tile([128, 1], F32, tag="bf")
            nc.sync.dma_start(out=bft, in_=biasf[:])
            ones_c = wp.tile([128, 1], F32, tag="oc")
            nc.vector.memset(ones_c, 1.0)
            ones_r = wp.tile([1, 128], F32, tag="or")
            nc.vector.memset(ones_r, 1.0)

            qst = qs.tile([128, NBLK, BLK], F32, tag="q")
            sq_slots = wp.tile([128, NBLK], F32, tag="sq")

            # ---- phase A: q = pc - pn/2 (9 taps, K=128), accumulate sum(q) ----
            blk = 0
            for img in range(IMGS):
                xt = x_t[img]
                for hb in range(H // ROWS_PER_BLK):
                    h0 = hb * ROWS_PER_BLK
                    qp = psq.tile([128, BLK], F32, tag="qp")
                    for t, (kh, kw) in enumerate(TAPS):
                        rhs = xt[:, h0 + kh:h0 + kh + ROWS_PER_BLK, kw:kw + W]
                        nc.tensor.matmul(
                            qp[:], qwt[:, t * F:(t + 1) * F], rhs,
                            start=(t == 0), stop=(t == 8),
                        )
                    nc.scalar.activation(
                        qst[:, blk, :], qp[:],
                        mybir.ActivationFunctionType.Copy,
                        accum_out=sq_slots[:, blk:blk + 1],
                    )
                    blk += 1

            # ---- gamma: local reduce -> AllReduce -> 1/(off + scale*S) ----
            sq_red = wp.tile([128, 1], F32, tag="sqr")
            nc.vector.reduce_sum(sq_red, sq_slots[:], axis=mybir.AxisListType.X)
            ps1 = pss.tile([1, 1], F32, tag="s1")
            nc.tensor.matmul(ps1[:], sq_red[:], ones_c[:],
                             start=True, stop=True)
            s_sb = wp.tile([1, 1], F32, tag="ssb")
            nc.scalar.copy(s_sb[:], ps1[:])
            cc_in = dr.tile([1, 1], F32, tag="cci")
            cc_out = dr.tile([1, 1], F32, tag="cco")
            nc.sync.dma_start(out=cc_in, in_=s_sb[:])
            nc.gpsimd.collective_compute(
                "AllReduce", mybir.AluOpType.add,
                replica_groups=[list(range(N_CORES))],
                ins=[cc_in.opt()], outs=[cc_out.opt()],
            )
            stot = wp.tile([1, 1], F32, tag="stot")
            nc.sync.dma_start(out=stot, in_=cc_out)
            den = wp.tile([1, 1], F32, tag="den")
            nc.vector.tensor_scalar(
                out=den[:], in0=stot[:],
                scalar1=float(scale_const), scalar2=float(offset_const),
                op0=mybir.AluOpType.mult, op1=mybir.AluOpType.add,
            )
            gam = wp.tile([1, 1], F32, tag="gam")
            nc.vector.reciprocal(gam[:], den[:])
            psg = pss.tile([128, 1], F32, tag="pg")
            nc.tensor.matmul(psg[:], ones_r[:], gam[:],
                             start=True, stop=True)
            scal = wp.tile([128, 1], F32, tag="scal")
            nc.vector.tensor_scalar_mul(scal[:], psg[:], 2.0)
            bias_g = wp.tile([128, 1], F32, tag="bg")
            nc.vector.tensor_mul(bias_g[:], psg[:], cnn[:])

            # ---- phase B: conv (9 taps, K=64) + exp + add + store ----
            blk = 0
            for img in range(IMGS):
                xt = x_t[img]
                for hb in range(H // ROWS_PER_BLK):
                    h0 = hb * ROWS_PER_BLK
                    cp = psc.tile([128, BLK], F32, tag="cp")
                    for t, (kh, kw) in enumerate(TAPS):
                        rhs = xt[0:64, h0 + kh:h0 + kh + ROWS_PER_BLK, kw:kw + W]
                        nc.tensor.matmul(
                            cp[:], cw[:, t * F:(t + 1) * F], rhs,
                            start=(t == 0), stop=(t == 8),
                        )
                    kan = st.tile([128, BLK], F32, tag="kan")
                    nc.scalar.activation(
                        kan[:], qst[:, blk, :],
                        mybir.ActivationFunctionType.Exp,
                        bias=bias_g[:], scale=scal[:],
                    )
                    outt = st.tile([128, BLK], F32, tag="outt")
                    nc.vector.scalar_tensor_tensor(
                        out=outt[:], in0=kan[:], scalar=bft[:], in1=cp[:],
                        op0=mybir.AluOpType.add, op1=mybir.AluOpType.add,
                    )
                    nc.sync.dma_start(out=out[:, blk * BLK:(blk + 1) * BLK],
                                      in_=outt[:])
                    blk += 1

    nc.compile()
    return nc


def kernel(inputs, kernel, bias, control_points):
    global LAST_EXEC_TIME_NS
    x = np.ascontiguousarray(np.asarray(inputs, dtype=np.float32))
    kw_ = np.asarray(kernel, dtype=np.float32)
    bias = np.asarray(bias, dtype=np.float32)
    cp = np.asarray(control_points, dtype=np.float32)

    # weights: [kh,kw,C,F] -> [C, (kh*3+kw)*F + f]
    convw = np.ascontiguousarray(
        kw_.transpose(2, 0, 1, 3).reshape(C, 9 * F))
    cpw = cp.transpose(2, 0, 1, 3).reshape(C, 9 * F)
    qw = np.ascontiguousarray(
        np.concatenate([cpw, np.full((C, 9 * F), -0.5, np.float32)], axis=0))

    cn = (cp.reshape(KH * KW * C, F).astype(np.float64) ** 2).sum(axis=0)
    offset_const = float(2.0 * cn.sum() / F)
    scale_const = float(-4.0 / (NTOT * F))
    cnneg = np.ascontiguousarray(-cn.astype(np.float32).reshape(F, 1))
    biasf = np.ascontiguousarray(bias.reshape(F, 1))

    in_maps = []
    for core in range(N_CORES):
        xs = x[core * IMGS:(core + 1) * IMGS]          # [4,64,64,64]
        xt = xs.transpose(3, 0, 1, 2)                  # [C,4,64,64]
        xpad = np.zeros((C, IMGS, HP, WP), np.float32)
        xpad[:, :, 1:H + 1, 1:W + 1] = xt
        xxc = np.ascontiguousarray(
            np.concatenate([xpad, xpad * xpad], axis=0))  # [128,4,66,66]
        in_maps.append({
            "xx": xxc, "convw": convw, "qw": qw,
            "cnneg": cnneg, "biasf": biasf,
        })

    nc = _build(offset_const, scale_const)
    res = run_bass_kernel_spmd(nc, in_maps, core_ids=list(range(N_CORES)))
    LAST_EXEC_TIME_NS = res.exec_time_ns

    out = np.empty((B, H, W, F), np.float32)
    for core in range(N_CORES):
        o = res.results[core]["out"]                   # [128, PIX]
        o = o.reshape(F, IMGS, H, W).transpose(1, 2, 3, 0)
        out[core * IMGS:(core + 1) * IMGS] = o
    return out


# revision 5
# speedup vs baseline: 1.1529x; 1.1529x over previous
"""KANConv2D Trainium2 kernel (8 NeuronCores, data-parallel over batch).

Math: out = conv(x, kernel) + exp(-gamma * d) + bias, where
  d[n,f]  = pn[n] + cn[f] - 2*pc[n,f]
  pc      = conv(x, control_points)      (patches @ control_points)
  pn[n]   = sum of x^2 over the 3x3xC patch
  gamma   = 1 / (2 * mean(d))            (global mean -> AllReduce)

Device strategy per core (4 images):
  - host pre-transposes/pads the shard to [C=64, img, 66, 66] and stacks x^2
    on SBUF partitions 64..127, so q := pc - pn/2 falls out of ONE 9-tap
    accumulated matmul group per 512-pixel block:
      lhsT_tap = [[cp_tap (64x128)], [-1/2 (64x128)]]  (K=128)
  - conv runs as its own 9-tap group (K=64); its results drain to a DRAM
    scratch so the PE keeps working through the AllReduce latency window.
  - matmuls are emitted tap-outer over groups of 3 PSUM blocks and the
    walrus ldw-elision pass is enabled, so one LDWEIGHTS serves 3 matmuls.
  - matmuls use float32r (1-pass FP22 multiply, fp32 PSUM accumulate).
  - Sum(q) per core -> AllReduce -> gamma -> exp epilogue on ACT engine.
"""

import os
import sys

import numpy as np

for _p in ("/opt/trn_rl_repo", "/root/.axon_site/_ro/trn_rl_repo"):
    if os.path.isdir(_p) and _p not in sys.path:
        sys.path.insert(0, _p)

import concourse.bacc as bacc
import concourse.bass_utils as _bu
import concourse.tile as tile
from concourse import mybir
from concourse.bass_utils import run_bass_kernel_spmd


def _ensure_ntff_hook():
    """bass_utils imports antenv.axon_hooks when tracing under axon; this
    image's antenv lacks that module. Provide it and install the ctypes
    NTFF hook so BASS_TRACE=1 yields exec_time_ns."""
    import types
    try:
        from antenv.axon_hooks import get_axon_ntff_profile_hook  # noqa: F401
        return
    except ImportError:
        pass
    try:
        import antenv
        mod = types.ModuleType("antenv.axon_hooks")
        _state = {"hook": None}
        mod.set_axon_ntff_profile_hook = lambda h: _state.__setitem__("hook", h)
        mod.get_axon_ntff_profile_hook = lambda: _state["hook"]
        sys.modules["antenv.axon_hooks"] = mod
        antenv.axon_hooks = mod
        try:
            from trn_agent_boot.trn_boot import _ntff_profile_via_ctypes
            so = "/opt/axon/libaxon_pjrt.so"
            if os.path.exists(so):
                mod.set_axon_ntff_profile_hook(_ntff_profile_via_ctypes(so))
        except Exception:
            pass
    except Exception:
        pass


def _enable_ldw_opt():
    """Consecutive matmuls sharing one weight tile only pay a single
    LDWEIGHTS if walrus's ldw-elision pass runs; concourse pins it off."""
    if getattr(_bu.run_command, "_ldw_patched", False):
        return
    orig = _bu.run_command

    def patched(argv, **kw):
        argv = ["--enable-ldw-opt=true" if a == "--enable-ldw-opt=false" else a
                for a in argv]
        return orig(argv, **kw)

    patched._ldw_patched = True
    _bu.run_command = patched


_ensure_ntff_hook()
_enable_ldw_opt()

B, H, W, C, F = 32, 64, 64, 64, 128
KH = KW = 3
N_CORES = 8
IMGS = B // N_CORES          # 4 images per core
HP, WP = H + 2, W + 2        # 66 padded
ROWS_PER_BLK = 8
BLK = ROWS_PER_BLK * W       # 512 pixels per block
BLKS_PER_IMG = H // ROWS_PER_BLK    # 8
NBLK = IMGS * BLKS_PER_IMG   # 32 blocks per core
PIX = IMGS * H * W           # 16384 pixels per core
NTOT = B * H * W             # 131072 pixels total

F32 = mybir.dt.float32
F32R = mybir.dt.float32r

TAPS = [(kh, kw) for kh in range(KH) for kw in range(KW)]
GROUPS = [(0, 1, 2), (3, 4, 5), (6, 7)]   # hb groups within an image

LAST_EXEC_TIME_NS = None


def _build(offset_const: float, scale_const: float):
    """offset_const = 2*sum(cn)/F ; scale_const = -4/(NTOT*F).
    gamma = 1 / (offset_const + scale_const * sum_q_total)."""
    nc = bacc.Bacc("TRN2", target_bir_lowering=False, debug=False,
                   num_devices=N_CORES)
    xx = nc.dram_tensor("xx", [128, IMGS, HP, WP], F32R, kind="ExternalInput")
    convw = nc.dram_tensor("convw", [64, 9 * F], F32R, kind="ExternalInput")
    qw = nc.dram_tensor("qw", [128, 9 * F], F32R, kind="ExternalInput")
    cnneg = nc.dram_tensor("cnneg", [128, 1], F32, kind="ExternalInput")
    biasf = nc.dram_tensor("biasf", [128, 1], F32, kind="ExternalInput")
    out = nc.dram_tensor("out", [128, PIX], F32, kind="ExternalOutput")

    with tile.TileContext(nc) as tc:
        with (
            tc.tile_pool(name="xp", bufs=1) as xp,
            tc.tile_pool(name="wp", bufs=1) as wp,
            tc.tile_pool(name="qs", bufs=1) as qs,
            tc.tile_pool(name="st", bufs=3) as st,
            tc.tile_pool(name="cs", bufs=6) as cs,
            tc.tile_pool(name="ps", bufs=6, space="PSUM") as ps,
            tc.tile_pool(name="pss", bufs=1, space="PSUM") as pss,
            tc.tile_pool(name="dr", bufs=1, space="DRAM") as dr,
        ):
            # ---- loads ----
            x_t = []
            for i in range(IMGS):
                t = xp.tile([128, HP, WP], F32R, tag=f"x{i}")
                nc.sync.dma_start(out=t, in_=xx[:, i])
                x_t.append(t)
            cw = wp.tile([64, 9 * F], F32R, tag="cw")
            nc.sync.dma_start(out=cw, in_=convw[:])
            qwt = wp.tile([128, 9 * F], F32R, tag="qw")
            nc.sync.dma_start(out=qwt, in_=qw[:])
            cnn = wp.tile([128, 1], F32, tag="cnn")
            nc.sync.dma_start(out=cnn, in_=cnneg[:])
            bft = wp.tile([128, 1], F32, tag="bf")
            nc.sync.dma_start(out=bft, in_=biasf[:])
            ones_c = wp.tile([128, 1], F32, tag="oc")
            nc.vector.memset(ones_c, 1.0)
            ones_r = wp.tile([1, 128], F32, tag="or")
            nc.vector.memset(ones_r, 1.0)

            qst = qs.tile([128, NBLK, BLK], F32, tag="q")
            sq_slots = wp.tile([128, NBLK], F32, tag="sq")

            # ---- phase A: q = pc - pn/2, tap-outer over groups of blocks ----
            for img in range(IMGS):
                xt = x_t[img]
                for grp in GROUPS:
                    qps = [ps.tile([128, BLK], F32, tag="mm", name=f"qp{img}_{hbx}") for hbx in grp]
                    for t, (kh, kw) in enumerate(TAPS):
                        wtile = qwt[:, t * F:(t + 1) * F]
                        for gi, hb in enumerate(grp):
                            h0 = hb * ROWS_PER_BLK
                            rhs = xt[:, h0 + kh:h0 + kh + ROWS_PER_BLK,
                                     kw:kw + W]
                            nc.tensor.matmul(qps[gi][:], wtile, rhs,
                                             start=(t == 0), stop=(t == 8))
                    for gi, hb in enumerate(grp):
                        blk = img * BLKS_PER_IMG + hb
                        nc.scalar.activation(
                            qst[:, blk, :], qps[gi][:],
                            mybir.ActivationFunctionType.Copy,
                            accum_out=sq_slots[:, blk:blk + 1],
                        )

            # ---- gamma: local reduce -> AllReduce -> 1/(off + scale*S) ----
            sq_red = wp.tile([128, 1], F32, tag="sqr")
            nc.vector.reduce_sum(sq_red, sq_slots[:], axis=mybir.AxisListType.X)
            ps1 = pss.tile([1, 1], F32, tag="s1")
            nc.tensor.matmul(ps1[:], sq_red[:], ones_c[:],
                             start=True, stop=True)
            s_sb = wp.tile([1, 1], F32, tag="ssb")
            nc.scalar.copy(s_sb[:], ps1[:])
            cc_in = dr.tile([1, 1], F32, tag="cci")
            cc_out = dr.tile([1, 1], F32, tag="cco")
            nc.sync.dma_start(out=cc_in, in_=s_sb[:])
            nc.gpsimd.collective_compute(
                "AllReduce", mybir.AluOpType.add,
                replica_groups=[list(range(N_CORES))],
                ins=[cc_in.opt()], outs=[cc_out.opt()],
            )
            stot = wp.tile([1, 1], F32, tag="stot")
            nc.sync.dma_start(out=stot, in_=cc_out)
            den = wp.tile([1, 1], F32, tag="den")
            nc.vector.tensor_scalar(
                out=den[:], in0=stot[:],
                scalar1=float(scale_const), scalar2=float(offset_const),
                op0=mybir.AluOpType.mult, op1=mybir.AluOpType.add,
            )
            gam = wp.tile([1, 1], F32, tag="gam")
            nc.vector.reciprocal(gam[:], den[:])
            psg = pss.tile([128, 1], F32, tag="pg")
            nc.tensor.matmul(psg[:], ones_r[:], gam[:],
                             start=True, stop=True)
            scal = wp.tile([128, 1], F32, tag="scal")
            nc.vector.tensor_scalar_mul(scal[:], psg[:], 2.0)
            bias_g = wp.tile([128, 1], F32, tag="bg")
            nc.vector.tensor_mul(bias_g[:], psg[:], cnn[:])

            # ---- phase C: conv, drains to DRAM scratch (no gamma dep, so
            # the PE keeps running during the AllReduce) ----
            cdram = [dr.tile([128, H * W], F32, tag=f"cd{i}", name=f"cd{i}")
                     for i in range(IMGS)]
            for img in range(IMGS):
                xt = x_t[img]
                for grp in GROUPS:
                    cps = [ps.tile([128, BLK], F32, tag="mm", name=f"cp{img}_{hbx}") for hbx in grp]
                    for t, (kh, kw) in enumerate(TAPS):
                        wtile = cw[:, t * F:(t + 1) * F]
                        for gi, hb in enumerate(grp):
                            h0 = hb * ROWS_PER_BLK
                            rhs = xt[0:64, h0 + kh:h0 + kh + ROWS_PER_BLK,
                                     kw:kw + W]
                            nc.tensor.matmul(cps[gi][:], wtile, rhs,
                                             start=(t == 0), stop=(t == 8))
                    for gi, hb in enumerate(grp):
                        cst = cs.tile([128, BLK], F32, tag="cst")
                        nc.vector.tensor_copy(cst[:], cps[gi][:])
                        nc.sync.dma_start(
                            out=cdram[img][:, hb * BLK:(hb + 1) * BLK],
                            in_=cst[:])

            # ---- phase D: epilogue out = conv + exp(2g*q - g*cn) + bias ----
            for img in range(IMGS):
                for hb in range(BLKS_PER_IMG):
                    blk = img * BLKS_PER_IMG + hb
                    ct = st.tile([128, BLK], F32, tag="ct")
                    nc.sync.dma_start(
                        out=ct[:],
                        in_=cdram[img][:, hb * BLK:(hb + 1) * BLK])
                    kan = st.tile([128, BLK], F32, tag="kan")
                    nc.scalar.activation(
                        kan[:], qst[:, blk, :],
                        mybir.ActivationFunctionType.Exp,
                        bias=bias_g[:], scale=scal[:],
                    )
                    outt = st.tile([128, BLK], F32, tag="outt")
                    nc.vector.scalar_tensor_tensor(
                        out=outt[:], in0=kan[:], scalar=bft[:], in1=ct[:],
                        op0=mybir.AluOpType.add, op1=mybir.AluOpType.add,
                    )
                    nc.sync.dma_start(out=out[:, blk * BLK:(blk + 1) * BLK],
                                      in_=outt[:])

    nc.compile()
    return nc


def kernel(inputs, kernel, bias, control_points):
    global LAST_EXEC_TIME_NS
    x = np.ascontiguousarray(np.asarray(inputs, dtype=np.float32))
    kw_ = np.asarray(kernel, dtype=np.float32)
    bias = np.asarray(bias, dtype=np.float32)
    cp = np.asarray(control_points, dtype=np.float32)

    # weights: [kh,kw,C,F] -> [C, (kh*3+kw)*F + f]
    convw = np.ascontiguousarray(
        kw_.transpose(2, 0, 1, 3).reshape(C, 9 * F))
    cpw = cp.transpose(2, 0, 1, 3).reshape(C, 9 * F)
    qw = np.ascontiguousarray(
        np.concatenate([cpw, np.full((C, 9 * F), -0.5, np.float32)], axis=0))

    cn = (cp.reshape(KH * KW * C, F).astype(np.float64) ** 2).sum(axis=0)
    offset_const = float(2.0 * cn.sum() / F)
    scale_const = float(-4.0 / (NTOT * F))
    cnneg = np.ascontiguousarray(-cn.astype(np.float32).reshape(F, 1))
    biasf = np.ascontiguousarray(bias.reshape(F, 1))

    in_maps = []
    for core in range(N_CORES):
        xs = x[core * IMGS:(core + 1) * IMGS]          # [4,64,64,64]
        xt = xs.transpose(3, 0, 1, 2)                  # [C,4,64,64]
        xpad = np.zeros((C, IMGS, HP, WP), np.float32)
        xpad[:, :, 1:H + 1, 1:W + 1] = xt
        xxc = np.ascontiguousarray(
            np.concatenate([xpad, xpad * xpad], axis=0))  # [128,4,66,66]
        in_maps.append({
            "xx": xxc, "convw": convw, "qw": qw,
            "cnneg": cnneg, "biasf": biasf,
        })

    nc = _build(offset_const, scale_const)
    res = run_bass_kernel_spmd(nc, in_maps, core_ids=list(range(N_CORES)))
    LAST_EXEC_TIME_NS = res.exec_time_ns

    out = np.empty((B, H, W, F), np.float32)
    for core in range(N_CORES):
        o = res.results[core]["out"]                   # [128, PIX]
        o = o.reshape(F, IMGS, H, W).transpose(1, 2, 3, 0)
        out[core * IMGS:(core + 1) * IMGS] = o
    return out
